# revision 13
# baseline (speedup 1.0000x reference)
"""DGCNN forward on 8 Trainium2 NeuronCores (Bass/Tile).

- 1024 graphs sharded 8 ways (batch is sorted -> contiguous node ranges).
- GCN layer: u[t] = sum_{e: col=t} xt[row_e] + xt[t]; x' = tanh(dis_t*(u@W)+b),
  xt[s] = dis_s*x[s], dis = rsqrt(deg). Layer 1 uses T0[s] = dis_s*W0[z_s].
- Per-edge gathers: batched indirect DMA of 512B rows (fp16 hi|lo pairs for
  fp32-class accuracy -- sort-pool top-k is unstable otherwise).
- Scatter-add: one-hot matmuls accumulating in PSUM per 128-target window.
- Cross-core: AllGather of each layer's xt table.
- Sort-pool: iterative argmax (top-30 desc, stable); head via PE matmuls.

Host side: integer-only sharding/sorting/indexing. All float math on device.
"""
import sys
sys.path.insert(0, '/opt/trn_rl_repo')

import numpy as np
from contextlib import ExitStack

import concourse.bass as bass
from concourse import bacc, tile, mybir
from concourse.bass import IndirectOffsetOnAxis
from concourse import bass_utils

F32 = mybir.dt.float32
F16 = mybir.dt.float16
I32 = mybir.dt.int32
AX = mybir.AxisListType
OP = mybir.AluOpType
AF = mybir.ActivationFunctionType

NCORE = 8
FIN = 128
H = 128
KTOP = 30
C1 = 16
C2 = 32
KERN = 5
D = 3 * H
PL = (KTOP - KERN + 1) // 2      # 13
CONVL = KTOP - KERN + 1          # 26
EPS = 1e-5
VS = 256


# ----------------------------------------------------------------------------
def prep_inputs(z, edge_index, batch, B):
    z = np.asarray(z).astype(np.int32)
    row = np.asarray(edge_index[0]).astype(np.int64)
    col = np.asarray(edge_index[1]).astype(np.int64)
    batch = np.asarray(batch).astype(np.int64)
    GPC = B // NCORE

    counts = np.bincount(batch, minlength=B).astype(np.int64)
    starts = np.concatenate([[0], np.cumsum(counts)])
    node_start = starts[np.arange(NCORE) * GPC]
    node_end = starts[np.arange(NCORE) * GPC + GPC]
    ncnt = node_end - node_start
    LP = int(np.ceil(max(ncnt.max(), 128) / 128) * 128)
    NW = LP // 128
    assert counts.max() <= VS, f"graph too large: {counts.max()}"
    zero_win = min(int(np.ceil((LP - ncnt.min()) / 128)) + 1, NW)

    owner = np.searchsorted(node_end, row, side='right')
    tabrow = (owner * LP + (row - node_start[owner])).astype(np.int64)
    ecore = np.searchsorted(node_end, col, side='right')

    NCHW = 1
    per_core = []
    for c in range(NCORE):
        sel = ecore == c
        lcol = (col[sel] - node_start[c]).astype(np.int64)
        grow = tabrow[sel]
        win = lcol >> 7
        order = np.argsort(win, kind='stable')
        lcol, grow, win = lcol[order], grow[order], win[order]
        wcnt = np.bincount(win, minlength=NW)
        NCHW = max(NCHW, int(np.ceil(wcnt.max() / 128)))
        per_core.append((lcol, grow, win, wcnt))

    TOT = NW * NCHW
    in_maps = []
    for c in range(NCORE):
        lcol, grow, win, wcnt = per_core[c]
        wstart = np.concatenate([[0], np.cumsum(wcnt)])[:-1]
        pos = np.arange(len(lcol)) - wstart[win]
        slot = win * (NCHW * 128) + pos
        rowidx = np.zeros(TOT * 128, np.int32)
        colrel = np.full(TOT * 128, -1, np.int32)
        rowidx[slot] = grow.astype(np.int32)
        colrel[slot] = (lcol & 127).astype(np.int32)
        rowidx = np.ascontiguousarray(rowidx.reshape(TOT, 128).T)
        colrel = np.ascontiguousarray(colrel.reshape(TOT, 128).T)

        zl = np.zeros(LP, np.int32)
        zl[:ncnt[c]] = z[node_start[c]:node_end[c]]
        zloc = np.ascontiguousarray(zl.reshape(NW, 128).T)

        g0 = c * GPC
        in_maps.append(dict(
            rowidx=rowidx, colrel=colrel, zloc=zloc,
            cnts=np.ascontiguousarray(counts[g0:g0 + GPC].astype(np.int32)[:, None]),
            gsts=np.ascontiguousarray(
                (starts[g0:g0 + GPC] - node_start[c]).astype(np.int32)[:, None]),
        ))
    dims = dict(LP=LP, NW=NW, NCHW=NCHW, TOT=TOT, GPC=GPC, zero_win=zero_win)
    return in_maps, dims


def prep_weights(inp):
    """Pure layout transforms (no float math)."""
    f = lambda x: np.ascontiguousarray(np.asarray(x).astype(np.float32))
    w = {}
    for k in ('W0', 'W1', 'W2'):
        w[k] = f(inp[k])
    for k in ('b0', 'b1', 'b2'):
        w[k] = f(inp[k])[None, :]
    Wc1T = f(inp['Wc1']).T                       # [384,16]
    w['Wc1T'] = np.ascontiguousarray(Wc1T.reshape(3, 128, C1).transpose(1, 0, 2))
    w['bc1'] = f(inp['bc1'])[None, :]
    Wc2 = f(inp['Wc2'])                          # [32,16,5]
    w['Wc2f'] = np.ascontiguousarray(Wc2.transpose(2, 1, 0).reshape(KERN * C1, C2))
    w['bc2'] = f(inp['bc2'])[None, :]
    for k in ('g1', 'be1', 'rm1', 'rv1', 'g2', 'be2', 'rm2', 'rv2'):
        w[k] = f(inp[k])[:, None]
    Wl1 = f(inp['Wl1'])                          # [128, 32*13]
    w['Wl1r'] = np.ascontiguousarray(Wl1.reshape(128, C2, PL).transpose(1, 2, 0))  # [32,13,128]
    w['bl1'] = f(inp['bl1'])[None, :]
    w['Wl2'] = np.ascontiguousarray(f(inp['Wl2']).reshape(1, 128).T)
    w['bl2'] = f(inp['bl2']).reshape(1, 1)
    return w


# ----------------------------------------------------------------------------
def build_program(dims, debug=False, dbg_tap=False):
    LP, NW, NCHW, TOT, GPC = (dims['LP'], dims['NW'], dims['NCHW'],
                              dims['TOT'], dims['GPC'])
    ZW = dims['zero_win']
    GW = min(4 * NCHW, TOT)          # chunks per gather instruction

    nc = bacc.Bacc("TRN2", target_bir_lowering=False, debug=debug,
                   num_devices=NCORE)

    ins = {}
    def inp(name, shape, dt=F32):
        ins[name] = nc.dram_tensor(name, shape, dt, kind="ExternalInput")

    inp('rowidx', [128, TOT], I32)
    inp('colrel', [128, TOT], I32)
    inp('zloc', [128, NW], I32)
    inp('cnts', [GPC, 1], I32)
    inp('gsts', [GPC, 1], I32)
    WSHAPES = [('W0', [128, 128]), ('W1', [128, 128]), ('W2', [128, 128]),
               ('b0', [1, 128]), ('b1', [1, 128]), ('b2', [1, 128]),
               ('Wc1T', [128, 3, C1]), ('bc1', [1, C1]),
               ('Wc2f', [KERN * C1, C2]), ('bc2', [1, C2]),
               ('g1', [C1, 1]), ('be1', [C1, 1]), ('rm1', [C1, 1]), ('rv1', [C1, 1]),
               ('g2', [C2, 1]), ('be2', [C2, 1]), ('rm2', [C2, 1]), ('rv2', [C2, 1]),
               ('Wl1r', [C2, PL, 128]), ('bl1', [1, 128]),
               ('Wl2', [128, 1]), ('bl2', [1, 1])]
    for n, s in WSHAPES:
        inp(n, s)
    out_t = nc.dram_tensor('out', [1, GPC], F32, kind="ExternalOutput")
    dbg_t = (nc.dram_tensor('dbg', [LP, 128], F32, kind="ExternalOutput")
             if dbg_tap else None)

    with ExitStack() as ctx:
        tc = ctx.enter_context(tile.TileContext(nc))
        sb = ctx.enter_context(tc.tile_pool(name="sb", bufs=1))
        dram = ctx.enter_context(tc.tile_pool(name="dram", bufs=1, space="DRAM"))

        sh = [dram.tile([LP, 256], F16, tag=f"sh{i}", name=f"sh{i}")
              for i in range(3)]
        tab = [dram.tile([NCORE * LP, 256], F16, tag=f"tab{i}", name=f"tab{i}",
                         addr_space="Shared")
               for i in range(3)]
        xp = [dram.tile([LP, 128], F32, tag=f"xp{i}", name=f"xp{i}")
              for i in range(3)]
        x3l_d = dram.tile([LP + VS], F32, tag="x3l_d", name="x3l_d")
        dvT_d = dram.tile([LP], F32, tag="dvT_d", name="dvT_d")

        def stile(shape, dt, tag):
            return sb.tile(shape, dt, tag=tag, name=tag)

        # ---- constants ----
        iota_i = stile([128, 128], I32, "iota_i")
        nc.gpsimd.iota(iota_i[:], pattern=[[1, 128]], base=0, channel_multiplier=0)
        iota16 = stile([128, 128], F16, "iota16")
        nc.vector.tensor_copy(iota16[:], iota_i[:])
        pidx = stile([128, 1], I32, "pidx")
        nc.gpsimd.iota(pidx[:], pattern=[[1, 1]], base=0, channel_multiplier=1)
        id16 = stile([128, 128], F16, "id16")
        nc.vector.tensor_tensor(id16[:], iota_i[:], pidx[:].to_broadcast([128, 128]),
                                op=OP.is_equal)
        id32 = stile([128, 128], F32, "id32")
        nc.vector.tensor_copy(id32[:], id16[:])
        iotaV_i = stile([128, VS], I32, "iotaV_i")
        nc.gpsimd.iota(iotaV_i[:], pattern=[[1, VS]], base=0, channel_multiplier=0)
        iotaV = stile([128, VS], F32, "iotaV")
        nc.vector.tensor_copy(iotaV[:], iotaV_i[:])
        ones = stile([1, 512], F32, "ones")
        nc.vector.memset(ones[:], 1.0)
        ones16c = stile([128, 1], F16, "ones16c")
        nc.vector.memset(ones16c[:], 1.0)
        cb1 = stile([128, 1], F32, "cb1")
        nc.vector.memset(cb1[:], 1.0)
        cb15 = stile([128, 1], F32, "cb15")
        nc.vector.memset(cb15[:], 1.5)
        cbn30 = stile([128, 1], F32, "cbn30")
        nc.vector.memset(cbn30[:], -1e30)
        cbp30 = stile([128, 1], F32, "cbp30")
        nc.vector.memset(cbp30[:], 1e30)
        cbeps = stile([128, 1], F32, "cbeps")
        nc.vector.memset(cbeps[:], EPS)

        # ---- load inputs ----
        def load(name, shape, dt=F32):
            t = stile(shape, dt, f"ld_{name}")
            nc.sync.dma_start(t[:], ins[name].ap()[:])
            return t
        rowidx_sb = load('rowidx', [128, TOT], I32)
        colrel_i = load('colrel', [128, TOT], I32)
        zloc_i = load('zloc', [128, NW], I32)
        cnts_i = load('cnts', [GPC, 1], I32)
        gsts_i = load('gsts', [GPC, 1], I32)
        W = {n: load(n, s) for n, s in WSHAPES}

        colrel16 = stile([128, TOT], F32, "colrel16")
        nc.vector.tensor_copy(colrel16[:], colrel_i[:])
        zloc16 = stile([128, NW], F32, "zloc16")
        nc.vector.tensor_copy(zloc16[:], zloc_i[:])

        W0hi = stile([128, 128], F16, "W0hi")
        nc.vector.tensor_copy(W0hi[:], W['W0'][:])
        W0lo = stile([128, 128], F16, "W0lo")
        nc.vector.tensor_tensor(W0lo[:], W['W0'][:], W0hi[:], op=OP.subtract)

        b0ext = stile([1, 256], F32, "b0ext")
        nc.vector.memset(b0ext[:], 0.0)
        nc.vector.tensor_copy(b0ext[:, 0:128], W['b0'][:])

        # zero pad tails (avoid NaN poison)
        zt16 = stile([128, 256], F16, "zt16")
        nc.vector.memset(zt16[:], 0.0)
        for i, s in enumerate(sh):
            for w in range(NW - ZW, NW):
                nc.sync.dma_start(s[w * 128:(w + 1) * 128, :], zt16[:])
        ztv = stile([1, VS], F32, "ztv")
        nc.vector.memset(ztv[:], 0.0)
        nc.sync.dma_start(x3l_d[LP:LP + VS][None, :], ztv[:])

        wrk = ctx.enter_context(tc.tile_pool(name="wrk", bufs=3))
        gp = ctx.enter_context(tc.tile_pool(name="gp", bufs=2))

        def colhot_window(w):
            chv = wrk.tile([128, NCHW, 128], F16, tag="colhot", name="chv")
            for j in range(NCHW):
                k = w * NCHW + j
                nc.vector.tensor_scalar(
                    out=chv[:, j, :], in0=iota16[:],
                    scalar1=colrel16[:, k:k + 1], scalar2=None,
                    op0=OP.is_equal)
            return chv

        # ------------------- deg -> dis/disinv -------------------
        degall = stile([128, NW], F32, "degall")
        with tc.tile_pool(name="psD", bufs=2, space="PSUM") as psD:
            for w in range(NW):
                chv = colhot_window(w)
                dps = psD.tile([128, 2], F32, tag="degp", name="dps")
                for j in range(NCHW):
                    nc.tensor.matmul(dps[:, 0:1], lhsT=chv[:, j, :],
                                     rhs=ones16c[:],
                                     start=(j == 0), stop=(j == NCHW - 1))
                nc.scalar.activation(degall[:, w:w + 1], dps[:, 0:1],
                                     AF.Identity, bias=cb1[:], scale=1.0)
        dis = stile([128, NW], F32, "dis")
        dvv = stile([128, NW], F32, "dvv")
        t_s = stile([128, NW], F32, "t_s")
        nc.scalar.activation(t_s[:], degall[:], AF.Sqrt)
        r0 = stile([128, NW], F32, "r0")
        nc.vector.reciprocal(r0[:], t_s[:])
        t1 = stile([128, NW], F32, "t1n")
        nc.vector.tensor_tensor(t1[:], r0[:], r0[:], op=OP.mult)
        nc.vector.tensor_tensor(t1[:], t1[:], degall[:], op=OP.mult)
        nc.scalar.activation(t1[:], t1[:], AF.Identity, bias=cb15[:], scale=-0.5)
        nc.vector.tensor_tensor(dis[:], r0[:], t1[:], op=OP.mult)
        nc.vector.tensor_tensor(dvv[:], degall[:], dis[:], op=OP.mult)

        # disinv -> row layout [1, LP] via DRAM bounce
        with tc.tile_pool(name="psT", bufs=2, space="PSUM") as psT:
            for o in range(0, NW, 128):
                nn = min(128, NW - o)
                dvt_ps = psT.tile([128, 128], F32, tag="dvt", name="dvt_ps")
                nc.tensor.transpose(dvt_ps[0:nn, :], dvv[:, o:o + nn],
                                    identity=id32[:])
                dvt_sb = wrk.tile([128, 128], F32, tag="dvt_sb", name="dvt_sb")
                nc.scalar.copy(dvt_sb[0:nn, :], dvt_ps[0:nn, :])
                nc.sync.dma_start(
                    dvT_d[o * 128:(o + nn) * 128].rearrange("(w p) -> w p", p=128),
                    dvt_sb[0:nn, :])

        # ------------------- T0 build -------------------
        with tc.tile_pool(name="psT0", bufs=3, space="PSUM") as psT0:
            for w in range(NW):
                zh = wrk.tile([128, 128], F16, tag="zh", name="zh")
                nc.vector.tensor_scalar(out=zh[:], in0=iota16[:],
                                        scalar1=zloc16[:, w:w + 1], scalar2=None,
                                        op0=OP.is_equal)
                zhT_ps = psT0.tile([128, 128], F16, tag="zhT", name="zhT_ps")
                nc.tensor.transpose(zhT_ps[:], zh[:], identity=id16[:])
                zhT = wrk.tile([128, 128], F16, tag="zhTs", name="zhT")
                nc.scalar.copy(zhT[:], zhT_ps[:])
                t0ps = psT0.tile([128, 128], F32, tag="t0ps", name="t0ps")
                nc.tensor.matmul(t0ps[:], lhsT=zhT[:], rhs=W0hi[:],
                                 start=True, stop=False)
                nc.tensor.matmul(t0ps[:], lhsT=zhT[:], rhs=W0lo[:],
                                 start=False, stop=True)
                t0f = wrk.tile([128, 128], F32, tag="t0f", name="t0f")
                nc.scalar.activation(t0f[:], t0ps[:], AF.Copy,
                                     scale=dis[:, w:w + 1])
                xtw = wrk.tile([128, 256], F16, tag="xtw0", name="xtw")
                nc.vector.tensor_copy(xtw[:, 0:128], t0f[:])
                nc.vector.tensor_tensor(xtw[:, 128:256], t0f[:], xtw[:, 0:128],
                                        op=OP.subtract)
                nc.sync.dma_start(sh[0][w * 128:(w + 1) * 128, :], xtw[:])

        # ------------------- GCN layers -------------------
        x3l_st = stile([128, NW], F32, "x3l_st")

        def allgather(i):
            nc.gpsimd.collective_compute(
                "AllGather", OP.bypass,
                replica_groups=[list(range(NCORE))],
                ins=[sh[i][:].opt()], outs=[tab[i][:].opt()])

        GWIN = 16
        def layer(li, psL):
            Wn = (None, 'W1', 'W2')[li]
            bn = ('b0', 'b1', 'b2')[li]
            gts = {}
            dvTg = None
            for w in range(NW):
                if w % GWIN == 0:
                    dvTg = wrk.tile([1, GWIN * 128], F32, tag="dvTg",
                                    name="dvTg")
                    hi_n = min(GWIN * 128, LP - w * 128)
                    nc.sync.dma_start(
                        dvTg[:, 0:hi_n],
                        dvT_d[w * 128:w * 128 + hi_n][None, :])
                gi = (w * NCHW) // GW
                if (w * NCHW) % GW == 0:
                    k0 = gi * GW
                    gn = min(GW, TOT - k0)
                    gt = gp.tile([128, GW, 256], F16, tag="gt", name="gt")
                    nc.gpsimd.indirect_dma_start(
                        out=gt[:, 0:gn, :], out_offset=None,
                        in_=tab[li][:],
                        in_offset=IndirectOffsetOnAxis(
                            ap=rowidx_sb[:, k0:k0 + gn], axis=0))
                    gts[gi] = gt
                gt = gts[gi]
                chv = colhot_window(w)
                shw = wrk.tile([128, 256], F16, tag="shw", name="shw")
                nc.sync.dma_start(shw[:], sh[li][w * 128:(w + 1) * 128, :])

                if li == 0:
                    agg = psL.tile([128, 256], F32, tag="aggA", name="agg")
                    wo = (w % GWIN) * 128
                    nc.tensor.matmul(agg[:],
                                     lhsT=dvTg[:, wo:wo + 128],
                                     rhs=b0ext[:], start=True, stop=False)
                    nc.tensor.matmul(agg[:, 0:128], lhsT=id16[:],
                                     rhs=shw[:, 0:128], start=False, stop=False)
                    nc.tensor.matmul(agg[:, 128:256], lhsT=id16[:],
                                     rhs=shw[:, 128:256], start=False, stop=False)
                    for j in range(NCHW):
                        k = w * NCHW + j
                        jj = k - gi * GW
                        nc.tensor.matmul(agg[:], lhsT=chv[:, j, :],
                                         rhs=gt[:, jj, :],
                                         start=False, stop=(j == NCHW - 1))
                    us = wrk.tile([128, 128], F32, tag="us", name="us")
                    nc.scalar.copy(us[:], agg[:, 0:128])
                    nc.vector.tensor_tensor(us[:], us[:], agg[:, 128:256],
                                            op=OP.add)
                    xw = wrk.tile([128, 128], F32, tag="xw", name="xw")
                    nc.scalar.activation(xw[:], us[:], AF.Tanh,
                                         scale=dis[:, w:w + 1])
                else:
                    agg = psL.tile([128, 128], F32, tag="aggB", name="agg")
                    nc.tensor.matmul(agg[:], lhsT=shw[:, 0:128], rhs=id16[:],
                                     start=True, stop=False)
                    nc.tensor.matmul(agg[:], lhsT=shw[:, 128:256], rhs=id16[:],
                                     start=False, stop=False)
                    for j in range(NCHW):
                        k = w * NCHW + j
                        jj = k - gi * GW
                        nc.tensor.matmul(agg[:], lhsT=gt[:, jj, 0:128],
                                         rhs=chv[:, j, :],
                                         start=False, stop=False)
                        nc.tensor.matmul(agg[:], lhsT=gt[:, jj, 128:256],
                                         rhs=chv[:, j, :],
                                         start=False, stop=(j == NCHW - 1))
                    uT = wrk.tile([128, 128], F32, tag="uT", name="uT")
                    nc.scalar.copy(uT[:], agg[:])
                    if dbg_tap and li == 1:
                        nc.sync.dma_start(dbg_t.ap()[w * 128:(w + 1) * 128, :],
                                          uT[:])
                    vps = psL.tile([128, 128], F32, tag="vps", name="vps")
                    wo = (w % GWIN) * 128
                    nc.tensor.matmul(vps[:],
                                     lhsT=dvTg[:, wo:wo + 128],
                                     rhs=W[bn][:], start=True, stop=False)
                    nc.tensor.matmul(vps[:], lhsT=uT[:], rhs=W[Wn][:],
                                     start=False, stop=True)
                    xw = wrk.tile([128, 128], F32, tag="xw", name="xw")
                    nc.scalar.activation(xw[:], vps[:], AF.Tanh,
                                         scale=dis[:, w:w + 1])

                nc.sync.dma_start(xp[li][w * 128:(w + 1) * 128, :], xw[:])
                if li < 2:
                    xs = wrk.tile([128, 128], F32, tag="xs", name="xs")
                    nc.vector.tensor_scalar(out=xs[:], in0=xw[:],
                                            scalar1=dis[:, w:w + 1],
                                            scalar2=None, op0=OP.mult)
                    xtw = wrk.tile([128, 256], F16, tag="xtwL", name="xtw2")
                    nc.vector.tensor_copy(xtw[:, 0:128], xs[:])
                    nc.vector.tensor_tensor(xtw[:, 128:256], xs[:],
                                            xtw[:, 0:128], op=OP.subtract)
                    nc.sync.dma_start(sh[li + 1][w * 128:(w + 1) * 128, :],
                                      xtw[:])
                else:
                    nc.vector.tensor_copy(x3l_st[:, w:w + 1], xw[:, 127:128])

        with tc.tile_pool(name="psL", bufs=2, space="PSUM") as psL:
            allgather(0)
            layer(0, psL)
            allgather(1)
            layer(1, psL)
            allgather(2)
            layer(2, psL)

        nc.sync.dma_start(x3l_d[0:LP].rearrange("(w p) -> p w", p=128),
                          x3l_st[:, 0:NW])

        # ------------------- sort-pool + head -------------------
        pb = ctx.enter_context(tc.tile_pool(name="pb", bufs=1))
        def ptile(shape, dt, tag):
            return pb.tile(shape, dt, tag=tag, name=tag)

        vals_raw = ptile([GPC, VS], F32, "vals_raw")
        nc.gpsimd.indirect_dma_start(
            out=vals_raw[:], out_offset=None, in_=x3l_d[:][:, None],
            in_offset=IndirectOffsetOnAxis(ap=gsts_i[:, 0:1], axis=0))
        cntf = ptile([GPC, 1], F32, "cntf")
        nc.vector.tensor_copy(cntf[:], cnts_i[:])
        mask = ptile([GPC, VS], F32, "mask")
        nc.vector.tensor_tensor(mask[:], iotaV[0:GPC, :],
                                cntf[:].to_broadcast([GPC, VS]), op=OP.is_lt)
        pen = ptile([GPC, VS], F32, "pen")
        nc.scalar.activation(pen[:], mask[:], AF.Identity, bias=cbn30[0:GPC], scale=1e30)
        vals = ptile([GPC, VS], F32, "vals")
        nc.vector.tensor_tensor(vals[:], vals_raw[:], mask[:], op=OP.mult)
        nc.vector.tensor_tensor(vals[:], vals[:], pen[:], op=OP.add)

        selidx = ptile([GPC, KTOP], F32, "selidx")
        for j in range(KTOP):
            m = wrk.tile([GPC, 1], F32, tag="selm", name="m")
            nc.vector.tensor_reduce(m[:], vals[:], axis=AX.X, op=OP.max)
            eq = wrk.tile([GPC, VS], F32, tag="seleq", name="eq")
            nc.vector.tensor_tensor(eq[:], vals[:], m[:].to_broadcast([GPC, VS]),
                                    op=OP.is_equal)
            t2 = wrk.tile([GPC, VS], F32, tag="selt2", name="t2")
            nc.scalar.activation(t2[:], eq[:], AF.Identity, bias=cbp30[0:GPC],
                                 scale=-1e30)
            nc.vector.tensor_tensor(t2[:], t2[:], iotaV[0:GPC, :], op=OP.add)
            nc.vector.tensor_reduce(selidx[:, j:j + 1], t2[:], axis=AX.X,
                                    op=OP.min)
            oh = wrk.tile([GPC, VS], F32, tag="seloh", name="oh")
            nc.vector.tensor_tensor(oh[:], iotaV[0:GPC, :],
                                    selidx[:, j:j + 1].to_broadcast([GPC, VS]),
                                    op=OP.is_equal)
            nc.scalar.activation(oh[:], oh[:], AF.Copy, scale=-2e30)
            nc.vector.tensor_tensor(vals[:], vals[:], oh[:], op=OP.add)

        gstf = ptile([GPC, 1], F32, "gstf")
        nc.vector.tensor_copy(gstf[:], gsts_i[:])
        nidf = ptile([GPC, KTOP], F32, "nidf")
        nc.vector.tensor_scalar(out=nidf[:], in0=selidx[:],
                                scalar1=gstf[:, 0:1], scalar2=None, op0=OP.add)
        nid = ptile([GPC, KTOP], I32, "nid")
        nc.vector.tensor_copy(nid[:], nidf[:])
        pm = ptile([GPC, KTOP], F32, "pm")
        nc.vector.tensor_tensor(pm[:], iotaV[0:GPC, 0:KTOP],
                                cntf[:].to_broadcast([GPC, KTOP]), op=OP.is_lt)

        NT = KTOP * GPC
        CH = 512
        KPC = min(max(1, CH // GPC), KTOP)
        with tc.tile_pool(name="psP", bufs=2, space="PSUM") as psP:
            h1 = ptile([C1, NT], F32, "h1")
            for o in range(0, NT, KPC * GPC):
                k0 = o // GPC
                kn = min(KPC, KTOP - k0)
                wd = kn * GPC
                cps = psP.tile([C1, 512], F32, tag="cps", name="cps")
                nc.tensor.matmul(cps[:, 0:wd], lhsT=W['bc1'][:],
                                 rhs=ones[:, 0:wd], start=True, stop=False)
                for b in range(3):
                    ptc = pb.tile([GPC, KPC, 128], F32, tag=f"ptc{b}",
                                  name=f"ptc{b}", bufs=2)
                    nc.gpsimd.indirect_dma_start(
                        out=ptc[:, 0:kn, :], out_offset=None, in_=xp[b][:],
                        in_offset=IndirectOffsetOnAxis(ap=nid[:, k0:k0 + kn],
                                                       axis=0))
                    nc.vector.tensor_tensor(
                        ptc[:, 0:kn, :], ptc[:, 0:kn, :],
                        pm[:, k0:k0 + kn, None].to_broadcast([GPC, kn, 128]),
                        op=OP.mult)
                    Tbc = pb.tile([128, 512], F32, tag=f"Tbc{b}",
                                  name=f"Tbc{b}", bufs=2)
                    for k in range(kn):
                        tps = psP.tile([128, GPC], F32, tag="tps", name="tps")
                        nc.tensor.transpose(tps[:], ptc[:, k, :],
                                            identity=id32[0:GPC, 0:GPC])
                        nc.scalar.copy(Tbc[:, k * GPC:(k + 1) * GPC], tps[:])
                    nc.tensor.matmul(cps[:, 0:wd], lhsT=W['Wc1T'][:, b, :],
                                     rhs=Tbc[:, 0:wd], start=False,
                                     stop=(b == 2))
                nc.scalar.activation(h1[:, o:o + wd], cps[:, 0:wd], AF.Relu)
            s1 = ptile([C1, 1], F32, "s1")
            nc.scalar.activation(s1[:], W['rv1'][:], AF.Sqrt, bias=cbeps[0:C1])
            nc.vector.reciprocal(s1[:], s1[:])
            nc.vector.tensor_tensor(s1[:], s1[:], W['g1'][:], op=OP.mult)
            sf1 = ptile([C1, 1], F32, "sf1")
            nc.vector.tensor_tensor(sf1[:], W['rm1'][:], s1[:], op=OP.mult)
            nc.vector.tensor_tensor(sf1[:], W['be1'][:], sf1[:], op=OP.subtract)
            nc.scalar.activation(h1[:], h1[:], AF.Identity, bias=sf1[:, 0:1],
                                 scale=s1[:, 0:1])

            # conv2 rhs: hb [80, CONVL*GPC]
            NT2 = CONVL * GPC
            hb = ptile([KERN * C1, NT2], F32, "hb")
            h1v = h1[:].rearrange("c (k g) -> c k g", g=GPC)
            for q in range(KERN):
                nc.sync.dma_start(
                    hb[q * C1:(q + 1) * C1, :].rearrange("c (p g) -> c p g",
                                                         g=GPC),
                    h1v[:, q:q + CONVL, :])

            h2 = ptile([C2, NT2], F32, "h2")
            for o in range(0, NT2, 512):
                wd = min(512, NT2 - o)
                cps2 = psP.tile([C2, 512], F32, tag="cps2", name="cps2")
                nc.tensor.matmul(cps2[:, 0:wd], lhsT=W['bc2'][:],
                                 rhs=ones[:, 0:wd], start=True, stop=False)
                nc.tensor.matmul(cps2[:, 0:wd], lhsT=W['Wc2f'][:],
                                 rhs=hb[:, o:o + wd], start=False, stop=True)
                nc.scalar.activation(h2[:, o:o + wd], cps2[:, 0:wd], AF.Relu)
            s2 = ptile([C2, 1], F32, "s2")
            nc.scalar.activation(s2[:], W['rv2'][:], AF.Sqrt, bias=cbeps[0:C2])
            nc.vector.reciprocal(s2[:], s2[:])
            nc.vector.tensor_tensor(s2[:], s2[:], W['g2'][:], op=OP.mult)
            sf2 = ptile([C2, 1], F32, "sf2")
            nc.vector.tensor_tensor(sf2[:], W['rm2'][:], s2[:], op=OP.mult)
            nc.vector.tensor_tensor(sf2[:], W['be2'][:], sf2[:], op=OP.subtract)
            nc.scalar.activation(h2[:], h2[:], AF.Identity, bias=sf2[:, 0:1],
                                 scale=s2[:, 0:1])

            mp = ptile([C2, PL * GPC], F32, "mp")
            h2v = h2[:].rearrange("c (r two g) -> c r two g", two=2, g=GPC)
            mpv = mp[:].rearrange("c (r g) -> c r g", g=GPC)
            nc.vector.tensor_tensor(mpv, h2v[:, :, 0, :], h2v[:, :, 1, :],
                                    op=OP.max)

            lps = psP.tile([128, GPC], F32, tag="lps", name="lps", bufs=1)
            nc.tensor.matmul(lps[:], lhsT=W['bl1'][:], rhs=ones[:, 0:GPC],
                             start=True, stop=False)
            for r in range(PL):
                nc.tensor.matmul(lps[:], lhsT=W['Wl1r'][:, r, :],
                                 rhs=mp[:, r * GPC:(r + 1) * GPC],
                                 start=False, stop=(r == PL - 1))
            l1 = ptile([128, GPC], F32, "l1")
            nc.scalar.activation(l1[:], lps[:], AF.Relu)

            ops = psP.tile([1, GPC], F32, tag="opst", name="ops", bufs=1)
            nc.tensor.matmul(ops[:], lhsT=W['bl2'][:], rhs=ones[:, 0:GPC],
                             start=True, stop=False)
            nc.tensor.matmul(ops[:], lhsT=W['Wl2'][:], rhs=l1[:],
                             start=False, stop=True)
            o_sb = ptile([1, GPC], F32, "o_sb")
            nc.scalar.copy(o_sb[:], ops[:])
            nc.sync.dma_start(out_t.ap()[:], o_sb[:])

    nc.compile()
    return nc


# ----------------------------------------------------------------------------
def kernel(**inputs):
    B = 1024
    in_maps, dims = prep_inputs(inputs['z'], inputs['edge_index'],
                                inputs['batch'], B)
    w = prep_weights(inputs)
    for m in in_maps:
        m.update(w)
    nc = build_program(dims)
    res = bass_utils.run_bass_kernel_spmd(nc, in_maps,
                                          core_ids=list(range(NCORE)))
    out = np.concatenate([res.results[c]['out'][0] for c in range(NCORE)])
    return out.astype(np.float32)


# revision 14
# speedup vs baseline: 1.0909x; 1.0909x over previous
"""DGCNN forward on 8 Trainium2 NeuronCores (Bass/Tile).

- 1024 graphs sharded 8 ways (batch is sorted -> contiguous node ranges).
- GCN layer: u[t] = sum_{e: col=t} xt[row_e] + xt[t]; x' = tanh(dis_t*(u@W)+b),
  xt[s] = dis_s*x[s], dis = rsqrt(deg). Layer 1 uses T0[s] = dis_s*W0[z_s].
- Per-edge gathers: batched indirect DMA of 512B rows (fp16 hi|lo pairs for
  fp32-class accuracy -- sort-pool top-k is unstable otherwise).
- Scatter-add: one-hot matmuls accumulating in PSUM per 128-target window.
- Cross-core: AllGather of each layer's xt table.
- Sort-pool: iterative argmax (top-30 desc, stable); head via PE matmuls.

Host side: integer-only sharding/sorting/indexing. All float math on device.
"""
import sys
sys.path.insert(0, '/opt/trn_rl_repo')

import numpy as np
from contextlib import ExitStack

import concourse.bass as bass
from concourse import bacc, tile, mybir
from concourse.bass import IndirectOffsetOnAxis
from concourse import bass_utils

F32 = mybir.dt.float32
F16 = mybir.dt.float16
I32 = mybir.dt.int32
AX = mybir.AxisListType
OP = mybir.AluOpType
AF = mybir.ActivationFunctionType

NCORE = 8
FIN = 128
H = 128
KTOP = 30
C1 = 16
C2 = 32
KERN = 5
D = 3 * H
PL = (KTOP - KERN + 1) // 2      # 13
CONVL = KTOP - KERN + 1          # 26
EPS = 1e-5
VS = 256


# ----------------------------------------------------------------------------
def prep_inputs(z, edge_index, batch, B):
    z = np.asarray(z).astype(np.int32)
    row = np.asarray(edge_index[0]).astype(np.int64)
    col = np.asarray(edge_index[1]).astype(np.int64)
    batch = np.asarray(batch).astype(np.int64)
    GPC = B // NCORE

    counts = np.bincount(batch, minlength=B).astype(np.int64)
    starts = np.concatenate([[0], np.cumsum(counts)])
    node_start = starts[np.arange(NCORE) * GPC]
    node_end = starts[np.arange(NCORE) * GPC + GPC]
    ncnt = node_end - node_start
    LP = int(np.ceil(max(ncnt.max(), 128) / 128) * 128)
    NW = LP // 128
    assert counts.max() <= VS, f"graph too large: {counts.max()}"
    zero_win = min(int(np.ceil((LP - ncnt.min()) / 128)) + 1, NW)

    owner = np.searchsorted(node_end, row, side='right')
    tabrow = (owner * LP + (row - node_start[owner])).astype(np.int64)
    ecore = np.searchsorted(node_end, col, side='right')

    NCHW = 1
    per_core = []
    for c in range(NCORE):
        sel = ecore == c
        lcol = (col[sel] - node_start[c]).astype(np.int64)
        grow = tabrow[sel]
        win = lcol >> 7
        order = np.argsort(win, kind='stable')
        lcol, grow, win = lcol[order], grow[order], win[order]
        wcnt = np.bincount(win, minlength=NW)
        NCHW = max(NCHW, int(np.ceil(wcnt.max() / 128)))
        per_core.append((lcol, grow, win, wcnt))

    TOT = NW * NCHW
    in_maps = []
    for c in range(NCORE):
        lcol, grow, win, wcnt = per_core[c]
        wstart = np.concatenate([[0], np.cumsum(wcnt)])[:-1]
        pos = np.arange(len(lcol)) - wstart[win]
        slot = win * (NCHW * 128) + pos
        rowidx = np.zeros(TOT * 128, np.int32)
        colrel = np.full(TOT * 128, -1, np.int32)
        rowidx[slot] = grow.astype(np.int32)
        colrel[slot] = (lcol & 127).astype(np.int32)
        rowidx = np.ascontiguousarray(rowidx.reshape(TOT, 128).T)
        colrel = np.ascontiguousarray(colrel.reshape(TOT, 128).T)

        zl = np.zeros(LP, np.int32)
        zl[:ncnt[c]] = z[node_start[c]:node_end[c]]
        zloc = np.ascontiguousarray(zl.reshape(NW, 128).T)

        g0 = c * GPC
        in_maps.append(dict(
            rowidx=rowidx, colrel=colrel, zloc=zloc,
            cnts=np.ascontiguousarray(counts[g0:g0 + GPC].astype(np.int32)[:, None]),
            gsts=np.ascontiguousarray(
                (starts[g0:g0 + GPC] - node_start[c]).astype(np.int32)[:, None]),
        ))
    dims = dict(LP=LP, NW=NW, NCHW=NCHW, TOT=TOT, GPC=GPC, zero_win=zero_win)
    return in_maps, dims


def prep_weights(inp):
    """Pure layout transforms (no float math)."""
    f = lambda x: np.ascontiguousarray(np.asarray(x).astype(np.float32))
    w = {}
    for k in ('W0', 'W1', 'W2'):
        w[k] = f(inp[k])
    for k in ('b0', 'b1', 'b2'):
        w[k] = f(inp[k])[None, :]
    Wc1T = f(inp['Wc1']).T                       # [384,16]
    w['Wc1T'] = np.ascontiguousarray(Wc1T.reshape(3, 128, C1).transpose(1, 0, 2))
    w['bc1'] = f(inp['bc1'])[None, :]
    Wc2 = f(inp['Wc2'])                          # [32,16,5]
    w['Wc2f'] = np.ascontiguousarray(Wc2.transpose(2, 1, 0).reshape(KERN * C1, C2))
    w['bc2'] = f(inp['bc2'])[None, :]
    for k in ('g1', 'be1', 'rm1', 'rv1', 'g2', 'be2', 'rm2', 'rv2'):
        w[k] = f(inp[k])[:, None]
    Wl1 = f(inp['Wl1'])                          # [128, 32*13]
    w['Wl1r'] = np.ascontiguousarray(Wl1.reshape(128, C2, PL).transpose(1, 2, 0))  # [32,13,128]
    w['bl1'] = f(inp['bl1'])[None, :]
    w['Wl2'] = np.ascontiguousarray(f(inp['Wl2']).reshape(1, 128).T)
    w['bl2'] = f(inp['bl2']).reshape(1, 1)
    w['bc1c'] = f(inp['bc1'])[:, None]
    w['bc2c'] = f(inp['bc2'])[:, None]
    w['bl1c'] = f(inp['bl1'])[:, None]
    return w


# ----------------------------------------------------------------------------
def build_program(dims, debug=False, dbg_tap=False):
    LP, NW, NCHW, TOT, GPC = (dims['LP'], dims['NW'], dims['NCHW'],
                              dims['TOT'], dims['GPC'])
    ZW = dims['zero_win']
    GW = min(4 * NCHW, TOT)          # chunks per gather instruction

    nc = bacc.Bacc("TRN2", target_bir_lowering=False, debug=debug,
                   num_devices=NCORE)

    ins = {}
    def inp(name, shape, dt=F32):
        ins[name] = nc.dram_tensor(name, shape, dt, kind="ExternalInput")

    inp('rowidx', [128, TOT], I32)
    inp('colrel', [128, TOT], I32)
    inp('zloc', [128, NW], I32)
    inp('cnts', [GPC, 1], I32)
    inp('gsts', [GPC, 1], I32)
    WSHAPES = [('W0', [128, 128]), ('W1', [128, 128]), ('W2', [128, 128]),
               ('b0', [1, 128]), ('b1', [1, 128]), ('b2', [1, 128]),
               ('Wc1T', [128, 3, C1]), ('bc1', [1, C1]),
               ('Wc2f', [KERN * C1, C2]), ('bc2', [1, C2]),
               ('g1', [C1, 1]), ('be1', [C1, 1]), ('rm1', [C1, 1]), ('rv1', [C1, 1]),
               ('g2', [C2, 1]), ('be2', [C2, 1]), ('rm2', [C2, 1]), ('rv2', [C2, 1]),
               ('Wl1r', [C2, PL, 128]), ('bl1', [1, 128]),
               ('Wl2', [128, 1]), ('bl2', [1, 1]),
               ('bc1c', [C1, 1]), ('bc2c', [C2, 1]), ('bl1c', [128, 1])]
    for n, s in WSHAPES:
        inp(n, s)
    out_t = nc.dram_tensor('out', [1, GPC], F32, kind="ExternalOutput")
    dbg_t = (nc.dram_tensor('dbg', [LP, 128], F32, kind="ExternalOutput")
             if dbg_tap else None)

    with ExitStack() as ctx:
        tc = ctx.enter_context(tile.TileContext(nc))
        sb = ctx.enter_context(tc.tile_pool(name="sb", bufs=1))
        dram = ctx.enter_context(tc.tile_pool(name="dram", bufs=1, space="DRAM"))

        sh = [dram.tile([LP, 256], F16, tag=f"sh{i}", name=f"sh{i}")
              for i in range(3)]
        tab = [dram.tile([NCORE * LP, 256], F16, tag=f"tab{i}", name=f"tab{i}",
                         addr_space="Shared")
               for i in range(3)]
        xp = [dram.tile([LP, 128], F32, tag=f"xp{i}", name=f"xp{i}")
              for i in range(3)]
        x3l_d = dram.tile([LP + VS], F32, tag="x3l_d", name="x3l_d")

        def stile(shape, dt, tag):
            return sb.tile(shape, dt, tag=tag, name=tag)

        # ---- constants ----
        iota_i = stile([128, 128], I32, "iota_i")
        nc.gpsimd.iota(iota_i[:], pattern=[[1, 128]], base=0, channel_multiplier=0)
        iota16 = stile([128, 128], F16, "iota16")
        nc.vector.tensor_copy(iota16[:], iota_i[:])
        pidx = stile([128, 1], I32, "pidx")
        nc.gpsimd.iota(pidx[:], pattern=[[1, 1]], base=0, channel_multiplier=1)
        id16 = stile([128, 128], F16, "id16")
        nc.vector.tensor_tensor(id16[:], iota_i[:], pidx[:].to_broadcast([128, 128]),
                                op=OP.is_equal)
        id32 = stile([128, 128], F32, "id32")
        nc.vector.tensor_copy(id32[:], id16[:])
        iotaV_i = stile([128, VS], I32, "iotaV_i")
        nc.gpsimd.iota(iotaV_i[:], pattern=[[1, VS]], base=0, channel_multiplier=0)
        iotaV = stile([128, VS], F32, "iotaV")
        nc.vector.tensor_copy(iotaV[:], iotaV_i[:])
        ones = stile([1, 512], F32, "ones")
        nc.vector.memset(ones[:], 1.0)
        ones16c = stile([128, 1], F16, "ones16c")
        nc.vector.memset(ones16c[:], 1.0)
        cb1 = stile([128, 1], F32, "cb1")
        nc.vector.memset(cb1[:], 1.0)
        cb15 = stile([128, 1], F32, "cb15")
        nc.vector.memset(cb15[:], 1.5)
        cbn30 = stile([128, 1], F32, "cbn30")
        nc.vector.memset(cbn30[:], -1e30)
        cbp30 = stile([128, 1], F32, "cbp30")
        nc.vector.memset(cbp30[:], 1e30)
        cbeps = stile([128, 1], F32, "cbeps")
        nc.vector.memset(cbeps[:], EPS)

        # ---- load inputs ----
        def load(name, shape, dt=F32):
            t = stile(shape, dt, f"ld_{name}")
            nc.sync.dma_start(t[:], ins[name].ap()[:])
            return t
        rowidx_sb = load('rowidx', [128, TOT], I32)
        colrel_i = load('colrel', [128, TOT], I32)
        zloc_i = load('zloc', [128, NW], I32)
        cnts_i = load('cnts', [GPC, 1], I32)
        gsts_i = load('gsts', [GPC, 1], I32)
        W = {n: load(n, s) for n, s in WSHAPES}

        colrel16 = stile([128, TOT], F32, "colrel16")
        nc.vector.tensor_copy(colrel16[:], colrel_i[:])
        zloc16 = stile([128, NW], F32, "zloc16")
        nc.vector.tensor_copy(zloc16[:], zloc_i[:])

        W0hi = stile([128, 128], F16, "W0hi")
        nc.vector.tensor_copy(W0hi[:], W['W0'][:])
        W0lo = stile([128, 128], F16, "W0lo")
        nc.vector.tensor_tensor(W0lo[:], W['W0'][:], W0hi[:], op=OP.subtract)

        b0ext = stile([1, 256], F32, "b0ext")
        nc.vector.memset(b0ext[:], 0.0)
        nc.vector.tensor_copy(b0ext[:, 0:128], W['b0'][:])
        btmp_d = dram.tile([3, 256], F32, tag="btmp_d", name="btmp_d")
        nc.sync.dma_start(btmp_d[0:1, :], b0ext[:])
        nc.sync.dma_start(btmp_d[1:2, 0:128], W['b1'][:])
        nc.sync.dma_start(btmp_d[2:3, 0:128], W['b2'][:])
        brep0 = stile([128, 256], F32, "brep0")
        nc.sync.dma_start(brep0[:], btmp_d[0:1, :].to_broadcast([128, 256]))
        brep1 = stile([128, 128], F32, "brep1")
        nc.sync.dma_start(brep1[:], btmp_d[1:2, 0:128].to_broadcast([128, 128]))
        brep2 = stile([128, 128], F32, "brep2")
        nc.sync.dma_start(brep2[:], btmp_d[2:3, 0:128].to_broadcast([128, 128]))

        # zero pad tails (avoid NaN poison)
        zt16 = stile([128, 256], F16, "zt16")
        nc.vector.memset(zt16[:], 0.0)
        for i, s in enumerate(sh):
            for w in range(NW - ZW, NW):
                nc.sync.dma_start(s[w * 128:(w + 1) * 128, :], zt16[:])
        ztv = stile([1, VS], F32, "ztv")
        nc.vector.memset(ztv[:], 0.0)
        nc.sync.dma_start(x3l_d[LP:LP + VS][None, :], ztv[:])

        wrk = ctx.enter_context(tc.tile_pool(name="wrk", bufs=3))
        gp = ctx.enter_context(tc.tile_pool(name="gp", bufs=2))

        def colhot_window(w):
            chv = wrk.tile([128, NCHW, 128], F16, tag="colhot", name="chv")
            for j in range(NCHW):
                k = w * NCHW + j
                nc.vector.tensor_scalar(
                    out=chv[:, j, :], in0=iota16[:],
                    scalar1=colrel16[:, k:k + 1], scalar2=None,
                    op0=OP.is_equal)
            return chv

        # ------------------- deg -> dis/disinv -------------------
        degall = stile([128, NW], F32, "degall")
        with tc.tile_pool(name="psD", bufs=2, space="PSUM") as psD:
            for w in range(NW):
                chv = colhot_window(w)
                dps = psD.tile([128, 2], F32, tag="degp", name="dps")
                for j in range(NCHW):
                    nc.tensor.matmul(dps[:, 0:1], lhsT=chv[:, j, :],
                                     rhs=ones16c[:],
                                     start=(j == 0), stop=(j == NCHW - 1))
                nc.scalar.activation(degall[:, w:w + 1], dps[:, 0:1],
                                     AF.Identity, bias=cb1[:], scale=1.0)
        dis = stile([128, NW], F32, "dis")
        dvv = stile([128, NW], F32, "dvv")
        t_s = stile([128, NW], F32, "t_s")
        nc.scalar.activation(t_s[:], degall[:], AF.Sqrt)
        r0 = stile([128, NW], F32, "r0")
        nc.vector.reciprocal(r0[:], t_s[:])
        t1 = stile([128, NW], F32, "t1n")
        nc.vector.tensor_tensor(t1[:], r0[:], r0[:], op=OP.mult)
        nc.vector.tensor_tensor(t1[:], t1[:], degall[:], op=OP.mult)
        nc.scalar.activation(t1[:], t1[:], AF.Identity, bias=cb15[:], scale=-0.5)
        nc.vector.tensor_tensor(dis[:], r0[:], t1[:], op=OP.mult)
        nc.vector.tensor_tensor(dvv[:], degall[:], dis[:], op=OP.mult)

        # ------------------- T0 build -------------------
        with tc.tile_pool(name="psT0", bufs=3, space="PSUM") as psT0:
            for w in range(NW):
                zh = wrk.tile([128, 128], F16, tag="zh", name="zh")
                nc.vector.tensor_scalar(out=zh[:], in0=iota16[:],
                                        scalar1=zloc16[:, w:w + 1], scalar2=None,
                                        op0=OP.is_equal)
                zhT_ps = psT0.tile([128, 128], F16, tag="zhT", name="zhT_ps")
                nc.tensor.transpose(zhT_ps[:], zh[:], identity=id16[:])
                zhT = wrk.tile([128, 128], F16, tag="zhTs", name="zhT")
                nc.scalar.copy(zhT[:], zhT_ps[:])
                t0ps = psT0.tile([128, 128], F32, tag="t0ps", name="t0ps")
                nc.tensor.matmul(t0ps[:], lhsT=zhT[:], rhs=W0hi[:],
                                 start=True, stop=False)
                nc.tensor.matmul(t0ps[:], lhsT=zhT[:], rhs=W0lo[:],
                                 start=False, stop=True)
                t0f = wrk.tile([128, 128], F32, tag="t0f", name="t0f")
                nc.scalar.activation(t0f[:], t0ps[:], AF.Copy,
                                     scale=dis[:, w:w + 1])
                xtw = wrk.tile([128, 256], F16, tag="xtw0", name="xtw")
                nc.vector.tensor_copy(xtw[:, 0:128], t0f[:])
                nc.vector.tensor_tensor(xtw[:, 128:256], t0f[:], xtw[:, 0:128],
                                        op=OP.subtract)
                nc.sync.dma_start(sh[0][w * 128:(w + 1) * 128, :], xtw[:])

        # ------------------- GCN layers -------------------
        x3l_st = stile([128, NW], F32, "x3l_st")

        def allgather(i):
            nc.gpsimd.collective_compute(
                "AllGather", OP.bypass,
                replica_groups=[list(range(NCORE))],
                ins=[sh[i][:].opt()], outs=[tab[i][:].opt()])

        def layer(li, psL):
            Wn = (None, 'W1', 'W2')[li]
            brep = (brep0, brep1, brep2)[li]
            gts = {}
            for w in range(NW):
                gi = (w * NCHW) // GW
                if (w * NCHW) % GW == 0:
                    k0 = gi * GW
                    gn = min(GW, TOT - k0)
                    gt = gp.tile([128, GW, 256], F16, tag="gt", name="gt")
                    nc.gpsimd.indirect_dma_start(
                        out=gt[:, 0:gn, :], out_offset=None,
                        in_=tab[li][:],
                        in_offset=IndirectOffsetOnAxis(
                            ap=rowidx_sb[:, k0:k0 + gn], axis=0))
                    gts[gi] = gt
                gt = gts[gi]
                chv = colhot_window(w)
                shw = wrk.tile([128, 256], F16, tag="shw", name="shw")
                nc.sync.dma_start(shw[:], sh[li][w * 128:(w + 1) * 128, :])

                if li == 0:
                    agg = psL.tile([128, 256], F32, tag="aggA", name="agg")
                    dmat = wrk.tile([128, 128], F32, tag="dmat", name="dmat")
                    nc.vector.tensor_scalar(out=dmat[:], in0=id32[:],
                                            scalar1=dvv[:, w:w + 1],
                                            scalar2=None, op0=OP.mult)
                    nc.tensor.matmul(agg[:], lhsT=dmat[:], rhs=brep[:],
                                     start=True, stop=False)
                    nc.tensor.matmul(agg[:, 0:128], lhsT=id16[:],
                                     rhs=shw[:, 0:128], start=False, stop=False)
                    nc.tensor.matmul(agg[:, 128:256], lhsT=id16[:],
                                     rhs=shw[:, 128:256], start=False, stop=False)
                    for j in range(NCHW):
                        k = w * NCHW + j
                        jj = k - gi * GW
                        nc.tensor.matmul(agg[:], lhsT=chv[:, j, :],
                                         rhs=gt[:, jj, :],
                                         start=False, stop=(j == NCHW - 1))
                    us = wrk.tile([128, 128], F32, tag="us", name="us")
                    nc.scalar.copy(us[:], agg[:, 0:128])
                    nc.vector.tensor_tensor(us[:], us[:], agg[:, 128:256],
                                            op=OP.add)
                    xw = wrk.tile([128, 128], F32, tag="xw", name="xw")
                    nc.scalar.activation(xw[:], us[:], AF.Tanh,
                                         scale=dis[:, w:w + 1])
                else:
                    agg = psL.tile([128, 128], F32, tag="aggB", name="agg")
                    nc.tensor.matmul(agg[:], lhsT=shw[:, 0:128], rhs=id16[:],
                                     start=True, stop=False)
                    nc.tensor.matmul(agg[:], lhsT=shw[:, 128:256], rhs=id16[:],
                                     start=False, stop=False)
                    for j in range(NCHW):
                        k = w * NCHW + j
                        jj = k - gi * GW
                        nc.tensor.matmul(agg[:], lhsT=gt[:, jj, 0:128],
                                         rhs=chv[:, j, :],
                                         start=False, stop=False)
                        nc.tensor.matmul(agg[:], lhsT=gt[:, jj, 128:256],
                                         rhs=chv[:, j, :],
                                         start=False, stop=(j == NCHW - 1))
                    uT = wrk.tile([128, 128], F32, tag="uT", name="uT")
                    nc.scalar.copy(uT[:], agg[:])
                    if dbg_tap and li == 1:
                        nc.sync.dma_start(dbg_t.ap()[w * 128:(w + 1) * 128, :],
                                          uT[:])
                    vps = psL.tile([128, 128], F32, tag="vps", name="vps")
                    dmat = wrk.tile([128, 128], F32, tag="dmat", name="dmat")
                    nc.vector.tensor_scalar(out=dmat[:], in0=id32[:],
                                            scalar1=dvv[:, w:w + 1],
                                            scalar2=None, op0=OP.mult)
                    nc.tensor.matmul(vps[:], lhsT=dmat[:], rhs=brep[:],
                                     start=True, stop=False)
                    nc.tensor.matmul(vps[:], lhsT=uT[:], rhs=W[Wn][:],
                                     start=False, stop=True)
                    xw = wrk.tile([128, 128], F32, tag="xw", name="xw")
                    nc.scalar.activation(xw[:], vps[:], AF.Tanh,
                                         scale=dis[:, w:w + 1])

                nc.sync.dma_start(xp[li][w * 128:(w + 1) * 128, :], xw[:])
                if li < 2:
                    xs = wrk.tile([128, 128], F32, tag="xs", name="xs")
                    nc.vector.tensor_scalar(out=xs[:], in0=xw[:],
                                            scalar1=dis[:, w:w + 1],
                                            scalar2=None, op0=OP.mult)
                    xtw = wrk.tile([128, 256], F16, tag="xtwL", name="xtw2")
                    nc.vector.tensor_copy(xtw[:, 0:128], xs[:])
                    nc.vector.tensor_tensor(xtw[:, 128:256], xs[:],
                                            xtw[:, 0:128], op=OP.subtract)
                    nc.sync.dma_start(sh[li + 1][w * 128:(w + 1) * 128, :],
                                      xtw[:])
                else:
                    nc.vector.tensor_copy(x3l_st[:, w:w + 1], xw[:, 127:128])

        with tc.tile_pool(name="psL", bufs=2, space="PSUM") as psL:
            allgather(0)
            layer(0, psL)
            allgather(1)
            layer(1, psL)
            allgather(2)
            layer(2, psL)

        nc.sync.dma_start(x3l_d[0:LP].rearrange("(w p) -> p w", p=128),
                          x3l_st[:, 0:NW])

        # ------------------- sort-pool + head -------------------
        pb = ctx.enter_context(tc.tile_pool(name="pb", bufs=1))
        def ptile(shape, dt, tag):
            return pb.tile(shape, dt, tag=tag, name=tag)

        vals_raw = ptile([GPC, VS], F32, "vals_raw")
        nc.gpsimd.indirect_dma_start(
            out=vals_raw[:], out_offset=None, in_=x3l_d[:][:, None],
            in_offset=IndirectOffsetOnAxis(ap=gsts_i[:, 0:1], axis=0))
        cntf = ptile([GPC, 1], F32, "cntf")
        nc.vector.tensor_copy(cntf[:], cnts_i[:])
        mask = ptile([GPC, VS], F32, "mask")
        nc.vector.tensor_tensor(mask[:], iotaV[0:GPC, :],
                                cntf[:].to_broadcast([GPC, VS]), op=OP.is_lt)
        pen = ptile([GPC, VS], F32, "pen")
        nc.scalar.activation(pen[:], mask[:], AF.Identity, bias=cbn30[0:GPC], scale=1e30)
        vals = ptile([GPC, VS], F32, "vals")
        nc.vector.tensor_tensor(vals[:], vals_raw[:], mask[:], op=OP.mult)
        nc.vector.tensor_tensor(vals[:], vals[:], pen[:], op=OP.add)

        selidx = ptile([GPC, KTOP], F32, "selidx")
        for j in range(KTOP):
            m = wrk.tile([GPC, 1], F32, tag="selm", name="m")
            nc.vector.tensor_reduce(m[:], vals[:], axis=AX.X, op=OP.max)
            eq = wrk.tile([GPC, VS], F32, tag="seleq", name="eq")
            nc.vector.tensor_tensor(eq[:], vals[:], m[:].to_broadcast([GPC, VS]),
                                    op=OP.is_equal)
            t2 = wrk.tile([GPC, VS], F32, tag="selt2", name="t2")
            nc.scalar.activation(t2[:], eq[:], AF.Identity, bias=cbp30[0:GPC],
                                 scale=-1e30)
            nc.vector.tensor_tensor(t2[:], t2[:], iotaV[0:GPC, :], op=OP.add)
            nc.vector.tensor_reduce(selidx[:, j:j + 1], t2[:], axis=AX.X,
                                    op=OP.min)
            oh = wrk.tile([GPC, VS], F32, tag="seloh", name="oh")
            nc.vector.tensor_tensor(oh[:], iotaV[0:GPC, :],
                                    selidx[:, j:j + 1].to_broadcast([GPC, VS]),
                                    op=OP.is_equal)
            nc.scalar.activation(oh[:], oh[:], AF.Copy, scale=-2e30)
            nc.vector.tensor_tensor(vals[:], vals[:], oh[:], op=OP.add)

        gstf = ptile([GPC, 1], F32, "gstf")
        nc.vector.tensor_copy(gstf[:], gsts_i[:])
        nidf = ptile([GPC, KTOP], F32, "nidf")
        nc.vector.tensor_scalar(out=nidf[:], in0=selidx[:],
                                scalar1=gstf[:, 0:1], scalar2=None, op0=OP.add)
        nid = ptile([GPC, KTOP], I32, "nid")
        nc.vector.tensor_copy(nid[:], nidf[:])
        pm = ptile([GPC, KTOP], F32, "pm")
        nc.vector.tensor_tensor(pm[:], iotaV[0:GPC, 0:KTOP],
                                cntf[:].to_broadcast([GPC, KTOP]), op=OP.is_lt)

        NT = KTOP * GPC
        CH = 512
        KPC = min(max(1, CH // GPC), KTOP)
        with tc.tile_pool(name="psP", bufs=2, space="PSUM") as psP:
            h1 = ptile([C1, NT], F32, "h1")
            for o in range(0, NT, KPC * GPC):
                k0 = o // GPC
                kn = min(KPC, KTOP - k0)
                wd = kn * GPC
                cps = psP.tile([C1, 512], F32, tag="cps", name="cps")
                for b in range(3):
                    ptc = pb.tile([GPC, KPC, 128], F32, tag=f"ptc{b}",
                                  name=f"ptc{b}", bufs=2)
                    nc.gpsimd.indirect_dma_start(
                        out=ptc[:, 0:kn, :], out_offset=None, in_=xp[b][:],
                        in_offset=IndirectOffsetOnAxis(ap=nid[:, k0:k0 + kn],
                                                       axis=0))
                    nc.vector.tensor_tensor(
                        ptc[:, 0:kn, :], ptc[:, 0:kn, :],
                        pm[:, k0:k0 + kn, None].to_broadcast([GPC, kn, 128]),
                        op=OP.mult)
                    Tbc = pb.tile([128, 512], F32, tag=f"Tbc{b}",
                                  name=f"Tbc{b}", bufs=2)
                    for k in range(kn):
                        tps = psP.tile([128, GPC], F32, tag="tps", name="tps")
                        nc.tensor.transpose(tps[:], ptc[:, k, :],
                                            identity=id32[0:GPC, 0:GPC])
                        nc.scalar.copy(Tbc[:, k * GPC:(k + 1) * GPC], tps[:])
                    nc.tensor.matmul(cps[:, 0:wd], lhsT=W['Wc1T'][:, b, :],
                                     rhs=Tbc[:, 0:wd], start=(b == 0),
                                     stop=(b == 2))
                nc.scalar.activation(h1[:, o:o + wd], cps[:, 0:wd], AF.Relu,
                                     bias=W['bc1c'][:])
            s1 = ptile([C1, 1], F32, "s1")
            nc.scalar.activation(s1[:], W['rv1'][:], AF.Sqrt, bias=cbeps[0:C1])
            nc.vector.reciprocal(s1[:], s1[:])
            nc.vector.tensor_tensor(s1[:], s1[:], W['g1'][:], op=OP.mult)
            sf1 = ptile([C1, 1], F32, "sf1")
            nc.vector.tensor_tensor(sf1[:], W['rm1'][:], s1[:], op=OP.mult)
            nc.vector.tensor_tensor(sf1[:], W['be1'][:], sf1[:], op=OP.subtract)
            nc.scalar.activation(h1[:], h1[:], AF.Identity, bias=sf1[:, 0:1],
                                 scale=s1[:, 0:1])

            # conv2 rhs: hb [80, CONVL*GPC]
            NT2 = CONVL * GPC
            hb = ptile([KERN * C1, NT2], F32, "hb")
            h1v = h1[:].rearrange("c (k g) -> c k g", g=GPC)
            for q in range(KERN):
                nc.sync.dma_start(
                    hb[q * C1:(q + 1) * C1, :].rearrange("c (p g) -> c p g",
                                                         g=GPC),
                    h1v[:, q:q + CONVL, :])

            h2 = ptile([C2, NT2], F32, "h2")
            for o in range(0, NT2, 512):
                wd = min(512, NT2 - o)
                cps2 = psP.tile([C2, 512], F32, tag="cps2", name="cps2")
                nc.tensor.matmul(cps2[:, 0:wd], lhsT=W['Wc2f'][:],
                                 rhs=hb[:, o:o + wd], start=True, stop=True)
                nc.scalar.activation(h2[:, o:o + wd], cps2[:, 0:wd], AF.Relu,
                                     bias=W['bc2c'][:])
            s2 = ptile([C2, 1], F32, "s2")
            nc.scalar.activation(s2[:], W['rv2'][:], AF.Sqrt, bias=cbeps[0:C2])
            nc.vector.reciprocal(s2[:], s2[:])
            nc.vector.tensor_tensor(s2[:], s2[:], W['g2'][:], op=OP.mult)
            sf2 = ptile([C2, 1], F32, "sf2")
            nc.vector.tensor_tensor(sf2[:], W['rm2'][:], s2[:], op=OP.mult)
            nc.vector.tensor_tensor(sf2[:], W['be2'][:], sf2[:], op=OP.subtract)
            nc.scalar.activation(h2[:], h2[:], AF.Identity, bias=sf2[:, 0:1],
                                 scale=s2[:, 0:1])

            mp = ptile([C2, PL * GPC], F32, "mp")
            h2v = h2[:].rearrange("c (r two g) -> c r two g", two=2, g=GPC)
            mpv = mp[:].rearrange("c (r g) -> c r g", g=GPC)
            nc.vector.tensor_tensor(mpv, h2v[:, :, 0, :], h2v[:, :, 1, :],
                                    op=OP.max)

            lps = psP.tile([128, GPC], F32, tag="lps", name="lps", bufs=1)
            for r in range(PL):
                nc.tensor.matmul(lps[:], lhsT=W['Wl1r'][:, r, :],
                                 rhs=mp[:, r * GPC:(r + 1) * GPC],
                                 start=(r == 0), stop=(r == PL - 1))
            l1 = ptile([128, GPC], F32, "l1")
            nc.scalar.activation(l1[:], lps[:], AF.Relu, bias=W['bl1c'][:])

            ops = psP.tile([1, GPC], F32, tag="opst", name="ops", bufs=1)
            nc.tensor.matmul(ops[:], lhsT=W['Wl2'][:], rhs=l1[:],
                             start=True, stop=True)
            o_sb = ptile([1, GPC], F32, "o_sb")
            nc.scalar.activation(o_sb[:], ops[:], AF.Identity,
                                 bias=W['bl2'][:, 0:1])
            nc.sync.dma_start(out_t.ap()[:], o_sb[:])

    nc.compile()
    return nc


# ----------------------------------------------------------------------------
def kernel(**inputs):
    B = 1024
    in_maps, dims = prep_inputs(inputs['z'], inputs['edge_index'],
                                inputs['batch'], B)
    w = prep_weights(inputs)
    for m in in_maps:
        m.update(w)
    nc = build_program(dims)
    res = bass_utils.run_bass_kernel_spmd(nc, in_maps,
                                          core_ids=list(range(NCORE)))
    out = np.concatenate([res.results[c]['out'][0] for c in range(NCORE)])
    return out.astype(np.float32)
